# revision 1
# baseline (speedup 1.0000x reference)
"""GCN decoder kernel for Trainium2, 8-core data-parallel over graphs.

Reference computation (per graph):
    a_hat = adj + I;  deg_j = sum_i a_hat[i,j];  d = rsqrt(deg)
    x = node_feat
    for l in 3 layers:
        h  = a_norm^T @ (x @ conv_w[l]) + conv_b[l]
        h  = h @ mlp_w[l] + mlp_b[l]
        x  = relu(layernorm(h))          # ln_g=1, ln_b=0
    mu = x @ lin_w + lin_b

Restructuring (exact algebra, host-side):
  - a_norm = d_i*(adj+I)*d_j precomputed on host, quantized to fp8e4/bf16.
  - conv_w[l] @ mlp_w[l] fused into W12[l] ((A^T x W1) W2 = (A^T x)(W1 W2)):
    each layer is ONE aggregation + ONE 128x128 matmul; x stays node-major
    the whole network -> no inter-layer transposes.
  - b2[l] = conv_b@mlp_w + mlp_b added via rank-1 (K=1) matmuls into the
    same PSUM accumulation group as the weight matmul.
  - a_norm scaled by 2^6, x0 by 2^4 (compensated exactly inside W12) to
    keep fp8e4m3 values out of the subnormal range.

Device schedule (per core, 2 graphs):
  - fp8 DoubleRow aggregation (x0 as exact hi+lo fp8 pair for layer 0;
    relu outputs quantized to fp8 for layers 1-2 in mode v3).
  - Aggregation accumulates into two [128,1024] PSUM tiles; per 1024-col
    half: 512-wide PSUM->SBUF bf16 copies split across DVE/ACT, 8 W12 +
    8 bias matmuls into two single-bank PSUM tiles, then LN stats via
    raw BNStats ops over an INTERLEAVED pair access pattern so the
    hardware's even/odd sub-accumulators yield exact per-block
    (count, mean, count*var) for two blocks per instruction (no merge
    ops); ReLU(LN) applied straight from PSUM on the scalar engine (one
    block per half on DVE as a 2-op apply for lane balance).
  - The two graphs' layers interleave in one stream so LN latency hides
    under the other graph's matmuls; adjacency streams in eighths so
    layer-0 aggregation paces behind the DMA.
  - Final linear: per-half 3D xbar DMA-transpose (node-major -> feature-
    major), 8 small matmuls + fused bias copy, one block-major output
    DMA per graph (host un-permutes); the very last half uses PE
    transposes since the PE is idle at the tail.
"""
import numpy as np

G, N, H, OUT, L = 16, 2048, 128, 64, 3
EPS = 1e-5
N_CORES = 8
GPC = G // N_CORES          # graphs per core
NB = N // 128               # 16 node blocks
NQ = 4                      # adjacency quarter tiles per graph
NH = 2                      # 1024-column halves

MODE = "v3"                 # "bf16" | "v2" | "v3"
STATS = "pair"              # "pair" (interleaved even/odd trick) | "block"
ADJ_SCALE = {"bf16": 1.0, "v2": 64.0, "v3": 64.0}[MODE]
X0_SCALE = {"bf16": 1.0, "v2": 16.0, "v3": 16.0}[MODE]

_cache = {}
MARKS = []


def _build(mode=MODE):
    import concourse.bass as bass
    import concourse.mybir as mybir
    import concourse.tile as tile
    from concourse import bacc

    f32 = mybir.dt.float32
    bf16 = mybir.dt.bfloat16
    fp8 = mybir.dt.float8e4
    Alu = mybir.AluOpType
    Act = mybir.ActivationFunctionType
    DR = mybir.MatmulPerfMode.DoubleRow

    adj_dt = bf16 if mode == "bf16" else fp8
    y_dt = fp8 if mode == "v3" else bf16

    nc = bacc.Bacc("TRN2", target_bir_lowering=False, debug=False,
                   num_devices=N_CORES)

    adjn_d = nc.dram_tensor("adjn", [GPC, N, N], adj_dt, kind="ExternalInput").ap()
    if mode == "bf16":
        x0_d = nc.dram_tensor("x0", [GPC, 128, N], bf16, kind="ExternalInput").ap()
    else:
        x0hi_d = nc.dram_tensor("x0hi", [GPC, 128, N], fp8, kind="ExternalInput").ap()
        x0lo_d = nc.dram_tensor("x0lo", [GPC, 128, N], fp8, kind="ExternalInput").ap()
    w12_d = nc.dram_tensor("w12", [128, L * H], bf16, kind="ExternalInput").ap()
    b2row_d = nc.dram_tensor("b2row", [1, L * H], bf16, kind="ExternalInput").ap()
    ones1_d = nc.dram_tensor("ones1", [1, 128], bf16, kind="ExternalInput").ap()
    identb_d = nc.dram_tensor("identb", [128, 128], bf16, kind="ExternalInput").ap()
    linw_d = nc.dram_tensor("lin_w", [128, OUT], bf16, kind="ExternalInput").ap()
    linbbc_d = nc.dram_tensor("linb_bc", [128, OUT], f32, kind="ExternalInput").ap()
    epsc_d = nc.dram_tensor("epsc", [128, 1], f32, kind="ExternalInput").ap()

    mu_d = nc.dram_tensor("mu", [GPC, 128, NB * OUT], f32,
                          kind="ExternalOutput").ap()

    with tile.TileContext(nc) as tc:
        with (
            tc.tile_pool(name="const", bufs=1) as cpool,
            tc.tile_pool(name="adjp", bufs=2 * NQ) as adjp,
            tc.tile_pool(name="act", bufs=1) as act,
            tc.tile_pool(name="small", bufs=4) as small,
            tc.tile_pool(name="psA", bufs=2, space="PSUM") as psA,
            tc.tile_pool(name="psM", bufs=2, space="PSUM") as psM,
        ):
            # ---- DMA plan, all on the sync (SP) queue, in arrival-need
            # order: x0(g0), adj(g0), consts, x0(g1), adj(g1) ----
            x0s, adjq = [], []

            def load_x0(g):
                if mode == "bf16":
                    x0 = act.tile([128, N], bf16, tag="y", bufs=4, name=f"x0_{g}")
                    nc.sync.dma_start(x0[:], x0_d[g])
                    x0s.append(x0)
                else:
                    xhi = act.tile([128, N], fp8, tag="xhi", bufs=2, name=f"x0hi_{g}")
                    xlo = act.tile([128, N], fp8, tag="xlo", bufs=2, name=f"x0lo_{g}")
                    nc.sync.dma_start(xhi[:], x0hi_d[g])
                    nc.sync.dma_start(xlo[:], x0lo_d[g])
                    x0s.append((xhi, xlo))


            ones1_t = cpool.tile([1, 128], bf16, name="ones1t")
            nc.gpsimd.dma_start(ones1_t[:], ones1_d)
            identb_t = cpool.tile([128, 128], bf16, name="identbt")
            nc.gpsimd.dma_start(identb_t[:], identb_d)
            load_x0(0)
            for g in range(GPC):
                adjq.append([adjp.tile([128, 4 * N], adj_dt, tag="adj",
                                       name=f"adj_{g}_{q}")
                             for q in range(NQ)])

            def adj_eighth_dma(g, e):
                q, eo = e // 2, e % 2
                nc.sync.dma_start(
                    adjq[g][q][:, eo * 2 * N:(eo + 1) * 2 * N].rearrange(
                        "p (i j) -> p i j", i=2),
                    adjn_d[g, q * 512 + eo * 256:
                           q * 512 + (eo + 1) * 256, :].rearrange(
                        "(i p) j -> p i j", p=128))

            for g, e in [(0, e) for e in range(2 * NQ)] + [
                    (1, e) for e in range(2 * NQ)]:
                adj_eighth_dma(g, e)
                if g == 0 and e == 4:
                    load_x0(1)
                if g == 0 and e == 5:
                    w12_t = cpool.tile([128, L * H], bf16, name="w12t")
                    nc.sync.dma_start(w12_t[:], w12_d)
                    b2row_t = cpool.tile([1, L * H], bf16, name="b2rowt")
                    nc.sync.dma_start(b2row_t[:], b2row_d)
                    eps_t = cpool.tile([128, 1], f32, name="epst")
                    nc.sync.dma_start(eps_t[:], epsc_d)
                    nc.const_aps.aps[(f32, EPS)] = eps_t[:]
                if g == 0 and e == 7:
                    linw_t = cpool.tile([128, OUT], bf16, name="linwt")
                    nc.sync.dma_start(linw_t[:], linw_d)
                    linb_t = cpool.tile([128, OUT], f32, name="linbt")
                    nc.sync.dma_start(linb_t[:], linbbc_d)

            def adj_ap(g, i, c):
                """[128, 512] slice for k-tile i, 512-column chunk c."""
                base = (i % 4) * N + c * 512
                return adjq[g][i // 4][:, base:base + 512]

            def adj_pair_ap(g, t, c):
                """[128, 2, 512] slice for k-tile pair (2t, 2t+1), chunk c."""
                q, p = t // 2, t % 2
                return adjq[g][q][:].rearrange("p (i j) -> p i j", i=4)[
                    :, 2 * p:2 * p + 2, c * 512:(c + 1) * 512]

            def agg_matmuls(g, l, c, y_ref):
                """[(lhsT, rhs, perf_mode), ...] accumulating 512-chunk c."""
                mms = []
                if mode == "bf16" or (mode == "v2" and l > 0):
                    src = y_ref if l > 0 else x0s[g]
                    for i in range(NB):
                        mms.append((src[:, i * 128:(i + 1) * 128],
                                    adj_ap(g, i, c), None))
                elif l == 0:
                    xhi, xlo = x0s[g]
                    for t in range(NB // 2):
                        for src in (xhi, xlo):
                            mms.append((
                                src[:, 2 * t * 128:(2 * t + 2) * 128].rearrange(
                                    "p (two k) -> p two k", two=2),
                                adj_pair_ap(g, t, c), DR))
                else:  # v3 layers 1-2: single fp8 pass in DoubleRow pairs
                    for t in range(NB // 2):
                        mms.append((
                            y_ref[:, 2 * t * 128:(2 * t + 2) * 128].rearrange(
                                "p (two k) -> p two k", two=2),
                            adj_pair_ap(g, t, c), DR))
                return mms

            # per-graph state
            st = [dict(y=None, aggps=[None] * NH, aggT=None, ynext=None,
                       istd=None, nbias=None, h2c=[None] * NH,
                       bn6=[None] * NH) for _ in range(GPC)]

            def alloc_layer(g, l):
                s = st[g]
                s["aggT"] = act.tile([128, N], bf16, tag="aggT", bufs=3,
                                     name=f"aggT_{g}_{l}")
                if l < L - 1:
                    s["ynext"] = act.tile([128, N], y_dt, tag="y", bufs=4,
                                          name=f"y_{g}_{l}")
                else:
                    s["ynext"] = act.tile([128, N], bf16, tag="x3", bufs=2,
                                          name=f"x3_{g}")
                s["istd"] = small.tile([128, NB], f32, tag="istd",
                                       name=f"istd_{g}_{l}")
                s["nbias"] = small.tile([128, NB], f32, tag="nbias",
                                        name=f"nbias_{g}_{l}")
                s["aggps"] = [psA.tile([128, 1024], f32, tag="agg",
                                       name=f"aggps_{g}_{l}_{h}")
                              for h in range(NH)]

            def emit_agg_half(g, l, h):
                s = st[g]
                per_chunk = [agg_matmuls(g, l, 2 * h + cg, s["y"])
                             for cg in range(2)]
                nk = len(per_chunk[0])
                for k in range(nk):
                    for cg in range(2):
                        lhsT, rhs, pm = per_chunk[cg][k]
                        nc.tensor.matmul(
                            s["aggps"][h][:, cg * 512:cg * 512 + 512],
                            lhsT, rhs, start=(k == 0), stop=(k == nk - 1),
                            perf_mode=pm)

            def emit_agg_paced(g, l):
                """All 4 column groups per k-step (layer 0 variant that
                tracks the arriving adjacency eighths; currently unused in
                the schedule — emit_agg_half paces equally well)."""
                s = st[g]
                per_chunk = [agg_matmuls(g, l, c, s["y"]) for c in range(4)]
                nk = len(per_chunk[0])
                for k in range(nk):
                    for c in range(4):
                        lhsT, rhs, pm = per_chunk[c][k]
                        nc.tensor.matmul(
                            s["aggps"][c // 2][:, (c % 2) * 512:
                                               (c % 2) * 512 + 512],
                            lhsT, rhs, start=(k == 0), stop=(k == nk - 1),
                            perf_mode=pm, skip_group_check=True)

            def emit_copies(g, l, h):
                """PSUM -> SBUF bf16, 512-wide, DVE and ACT in parallel."""
                s = st[g]
                base = h * 1024
                nc.scalar.copy(s["aggT"][:, base:base + 512],
                               s["aggps"][h][:, 0:512])
                nc.vector.tensor_copy(s["aggT"][:, base + 512:base + 1024],
                                      s["aggps"][h][:, 512:1024])

            def emit_h2_stats(g, l, h):
                """W12+bias matmuls into one PSUM tile + LN stats.

                STATS == "pair": one raw BNStats per block-pair with an
                interleaved access pattern, so the hardware's even/odd
                sub-accumulators yield EXACT per-block (count,mean,M2)
                without any merge.  STATS == "block": one BNStats + one
                bn_aggr per block (walrus-proven shapes)."""
                s = st[g]
                w = w12_t[:, l * H:(l + 1) * H]
                b2 = b2row_t[:, l * H:(l + 1) * H]
                h2q = [psM.tile([128, 512], f32, tag="h2a",
                                name=f"h2a_{g}_{l}_{h}"),
                       psM.tile([128, 512], f32, tag="h2b",
                                name=f"h2b_{g}_{l}_{h}")]
                s["h2c"][h] = h2q
                if STATS == "pair":
                    bn6 = small.tile([128, 4, 6], f32, tag="bn6",
                                     name=f"bn6_{g}_{l}_{h}")
                else:
                    bn6 = small.tile([128, 8, 6], f32, tag="bn6",
                                     name=f"bn6_{g}_{l}_{h}")
                s["bn6"][h] = bn6
                for q in range(2):          # one PSUM bank per 4 blocks
                    for jj in range(4):
                        j = 8 * h + 4 * q + jj
                        sl = slice(jj * 128, (jj + 1) * 128)
                        nc.tensor.matmul(
                            h2q[q][:, sl], s["aggT"][:, j * 128:(j + 1) * 128],
                            w, start=(jj == 0), stop=False,
                            skip_group_check=True)
                        nc.tensor.matmul(
                            h2q[q][:, sl], ones1_t[:], b2,
                            start=False, stop=(jj == 3),
                            skip_group_check=True)
                    if STATS == "pair":
                        for t in range(2):  # block pair (2t, 2t+1) within q
                            in_ap = h2q[q][:, 2 * t * 128:
                                           (2 * t + 2) * 128].rearrange(
                                "p (two k) -> p k two", two=2)
                            nc.vector.add_instruction(
                                mybir.InstBNStats(
                                    name=nc.get_next_instruction_name(),
                                    ins=[nc.vector.lower_ap(in_ap)],
                                    outs=[nc.vector.lower_ap(
                                        bn6[:, 2 * q + t, :])]))
                    else:
                        for jj in range(4):
                            nc.vector.bn_stats(
                                bn6[:, 4 * q + jj, :],
                                h2q[q][:, jj * 128:(jj + 1) * 128])

            def emit_chain(g, l, h):
                """LN stat chain -> istd, nbias columns."""
                s = st[g]
                bn6 = s["bn6"][h]
                slc = slice(8 * h, 8 * h + 8)
                stdv = small.tile([128, 8], f32, tag="stdv",
                                  name=f"stdv_{g}_{l}_{h}")
                if STATS == "pair":
                    # bn6[p, pair, (even triple, odd triple)]; triples are
                    # (count, mean, count*var); count == 128 per block.
                    tri = bn6[:].rearrange("p f (t s) -> p (f t) s", t=2)
                    means = tri[:, :, 1]
                    cvars = tri[:, :, 2]
                    nc.scalar.activation(stdv[:], cvars, Act.Sqrt,
                                         bias=EPS, scale=1.0 / H)
                    nc.vector.reciprocal(s["istd"][:, slc], stdv[:])
                    nc.vector.scalar_tensor_tensor(
                        out=s["nbias"][:, slc], in0=means, scalar=-1.0,
                        in1=s["istd"][:, slc], op0=Alu.mult, op1=Alu.mult)
                else:
                    mvt = small.tile([128, 8, 2], f32, tag="mvt",
                                     name=f"mvt_{g}_{l}_{h}")
                    for jj in range(8):
                        nc.vector.bn_aggr(mvt[:, jj, :], bn6[:, jj, :])
                    nc.scalar.activation(stdv[:], mvt[:, :, 1], Act.Sqrt,
                                         bias=EPS)
                    nc.vector.reciprocal(s["istd"][:, slc], stdv[:])
                    nc.vector.scalar_tensor_tensor(
                        out=s["nbias"][:, slc], in0=mvt[:, :, 0], scalar=-1.0,
                        in1=s["istd"][:, slc], op0=Alu.mult, op1=Alu.mult)

            def emit_applies(g, l, h):
                """ReLU(LN) from PSUM -> y_next, all on the scalar engine."""
                s = st[g]
                h2q = s["h2c"][h]
                for jj in range(8):
                    j = 8 * h + jj
                    hsl = h2q[jj // 4][:, (jj % 4) * 128:(jj % 4 + 1) * 128]
                    ysl = s["ynext"][:, j * 128:(j + 1) * 128]
                    if jj == 7:    # one per half on DVE (2-op apply)
                        tmp = small.tile([128, 128], f32, tag="ptmp",
                                         name=f"ptmp_{g}_{l}_{h}_{jj}",
                                         bufs=4)
                        nc.vector.tensor_scalar(
                            tmp[:], hsl, s["istd"][:, j:j + 1],
                            s["nbias"][:, j:j + 1],
                            op0=Alu.mult, op1=Alu.add)
                        nc.vector.tensor_scalar(
                            ysl, tmp[:], 0.0, None, op0=Alu.max)
                    else:
                        nc.scalar.activation(
                            ysl, hsl, Act.Relu,
                            bias=s["nbias"][:, j:j + 1],
                            scale=s["istd"][:, j:j + 1])

            def emit_post(g, l, h):
                emit_h2_stats(g, l, h)
                emit_chain(g, l, h)
                emit_applies(g, l, h)

            def emit_post_both(g, l, mark_pref=None):
                """Both halves with stage-level interleave so neither
                half's stat chain queues behind the other's applies."""
                emit_post(g, l, 0)
                if mark_pref:
                    mark(f"{mark_pref}h0")
                emit_post(g, l, 1)
                if mark_pref:
                    mark(f"{mark_pref}h1")

            def finish_layer(g):
                st[g]["y"] = st[g]["ynext"]

            def emit_final_half(g, x3, xT, musb, h, pe_tr=False):
                if pe_tr:
                    # tail path: PE transposes (PE is idle here) avoid the
                    # xbar-DMA latency on the critical last half
                    for jj in range(8):
                        j = 8 * h + jj
                        sl = slice(j * 128, (j + 1) * 128)
                        trp = psA.tile([128, 128], bf16, tag="agg",
                                       name=f"trp_{g}_{j}")
                        nc.tensor.transpose(trp[:], x3[:, sl], identb_t[:])
                        if jj % 2 == 0:
                            nc.vector.tensor_copy(xT[:, sl], trp[:])
                        else:
                            nc.scalar.copy(xT[:, sl], trp[:])
                else:
                    nc.sync.dma_start_transpose(
                        xT[:].rearrange("p (b q) -> p b q", b=NB)[
                            :, 8 * h:8 * h + 8, :],
                        x3[:, h * 1024:(h + 1) * 1024])
                for jj in range(8):
                    j = 8 * h + jj
                    sl = slice(j * 128, (j + 1) * 128)
                    mup = psA.tile([128, OUT], f32, tag="agg",
                                   name=f"mup_{g}_{j}")
                    nc.tensor.matmul(mup[:], xT[:, sl], linw_t[:],
                                     start=True, stop=True)
                    osl = slice(j * OUT, (j + 1) * OUT)
                    nc.vector.scalar_tensor_tensor(
                        out=musb[:, osl], in0=mup[:], scalar=1.0,
                        in1=linb_t[:], op0=Alu.mult, op1=Alu.add)
                if h == NH - 1:     # one mu DMA per graph; the tail
                    # graph uses the idle ACT hwdge queue
                    dq = nc.scalar if g == 1 else nc.sync
                    dq.dma_start(mu_d[g], musb[:])

            # ---- symmetric alternating two-graph schedule ----
            MARKS.clear()

            def mark(label):
                MARKS.append((label, list(nc.all_instructions())[-1].name))

            def emit_layer_aggs(g, l, paced=False, warmup=False):
                alloc_layer(g, l)
                if paced:
                    emit_agg_paced(g, l)
                    emit_copies(g, l, 0)
                    emit_copies(g, l, 1)
                else:
                    emit_agg_half(g, l, 0)
                    emit_copies(g, l, 0)
                    emit_agg_half(g, l, 1)
                    emit_copies(g, l, 1)
                mark(f"agg g{g}l{l}")

            def emit_layer_posts(g, l):
                emit_post_both(g, l, mark_pref=f"post g{g}l{l}")
                finish_layer(g)

            # layer-0 pair: DMA-skewed, posts per graph
            emit_layer_aggs(0, 0)
            emit_layer_posts(0, 0)
            emit_layer_aggs(1, 0)
            emit_layer_posts(1, 0)
            # layer-1 pair: aggs back-to-back, stage-merged posts
            emit_layer_aggs(0, 1)
            emit_layer_aggs(1, 1)
            for g in range(GPC):
                emit_post_both(g, 1, mark_pref=f"post g{g}l1")
            for g in range(GPC):
                finish_layer(g)
            # layer-2: per-graph staggered posts + finals
            emit_layer_aggs(0, 2)
            emit_layer_aggs(1, 2)
            for g in range(GPC):
                s = st[g]
                xT = act.tile([128, N], bf16, tag="xT", bufs=2, name=f"xT_{g}")
                musb = act.tile([128, NB * OUT], f32, tag="mu", bufs=2,
                                name=f"musb_{g}")
                x3 = s["ynext"]
                emit_post(g, 2, 0)
                mark(f"post g{g}l2h0")
                emit_final_half(g, x3, xT, musb, 0)
                mark(f"final g{g}h0")
                emit_post(g, 2, 1)
                mark(f"post g{g}l2h1")
                emit_final_half(g, x3, xT, musb, 1, pe_tr=(g == 1))
                mark(f"final g{g}h1")
                finish_layer(g)

    nc.compile()
    return nc


def kernel(node_feat, adj, conv_w, conv_b, mlp_w, mlp_b, ln_g, ln_b, lin_w,
           lin_b, **_ignored):
    import ml_dtypes
    from concourse.bass_utils import run_bass_kernel_spmd

    bf16 = ml_dtypes.bfloat16
    fp8 = ml_dtypes.float8_e4m3

    node_feat = np.asarray(node_feat, dtype=np.float32)
    adj = np.asarray(adj, dtype=np.float32)
    conv_w = np.asarray(conv_w, dtype=np.float32)
    conv_b = np.asarray(conv_b, dtype=np.float32)
    mlp_w = np.asarray(mlp_w, dtype=np.float32)
    mlp_b = np.asarray(mlp_b, dtype=np.float32)
    ln_g = np.asarray(ln_g, dtype=np.float32)
    ln_b = np.asarray(ln_b, dtype=np.float32)
    lin_w = np.asarray(lin_w, dtype=np.float32)
    lin_b = np.asarray(lin_b, dtype=np.float32)

    assert np.allclose(ln_g, 1.0) and np.allclose(ln_b, 0.0), \
        "kernel specialized for ln_g=1, ln_b=0 (as produced by setup_inputs)"

    if "nc" not in _cache:
        _cache["nc"] = _build()
    nc = _cache["nc"]

    # ---- host-side exact preprocessing ----
    deg = 1.0 + adj.sum(axis=1)                      # [G, N]
    d = deg ** -0.5
    adj_dt = bf16 if MODE == "bf16" else fp8
    adjn = np.empty((G, N, N), dtype=adj_dt)
    idx = np.arange(N)
    for g in range(G):
        an = adj[g] * (ADJ_SCALE * d[g][:, None] * d[g][None, :])
        an[idx, idx] += ADJ_SCALE * d[g] * d[g]
        adjn[g] = an.astype(adj_dt)

    # x0 in node-block layout [g, p, (i k)]: node (i*128+p) -> [p, i*H+k]
    x0 = node_feat.reshape(G, NB, 128, H).transpose(0, 2, 1, 3).reshape(
        G, 128, N) * X0_SCALE
    w12 = np.einsum('lhx,lxk->lhk', conv_w, mlp_w)
    w12[0] /= (ADJ_SCALE * X0_SCALE)
    w12[1] /= ADJ_SCALE
    w12[2] /= ADJ_SCALE
    w12_t = np.ascontiguousarray(
        w12.transpose(1, 0, 2).reshape(H, L * H)).astype(bf16)
    b2 = np.einsum('lh,lhk->lk', conv_b, mlp_w) + mlp_b        # [L, H]
    b2row = b2.reshape(1, L * H).astype(bf16)
    ones1 = np.ones((1, 128), dtype=bf16)
    identb = np.eye(128, dtype=np.float32).astype(bf16)
    linw = lin_w.astype(bf16)
    linb_bc = np.ascontiguousarray(
        np.broadcast_to(lin_b[None, :], (128, OUT))).astype(np.float32)
    epsc = np.full((128, 1), EPS, dtype=np.float32)

    in_maps = []
    for c in range(N_CORES):
        m = {
            "adjn": np.ascontiguousarray(adjn[c * GPC:(c + 1) * GPC]),
            "w12": w12_t, "b2row": b2row, "ones1": ones1, "identb": identb,
            "lin_w": linw, "linb_bc": linb_bc, "epsc": epsc,
        }
        xs = x0[c * GPC:(c + 1) * GPC]
        if MODE == "bf16":
            m["x0"] = np.ascontiguousarray(xs.astype(bf16))
        else:
            hi = xs.astype(fp8)
            lo = (xs - hi.astype(np.float32)).astype(fp8)
            m["x0hi"] = np.ascontiguousarray(hi)
            m["x0lo"] = np.ascontiguousarray(lo)
        in_maps.append(m)

    res = run_bass_kernel_spmd(nc, in_maps, core_ids=list(range(N_CORES)),
                               **_cache.get("run_kwargs", {}))
    _cache["last_result"] = res
    mu_blk = np.concatenate([res.results[c]["mu"] for c in range(N_CORES)],
                            axis=0)                      # [G, 128, NB*OUT]
    mu = np.ascontiguousarray(
        mu_blk.reshape(G, 128, NB, OUT).transpose(0, 2, 1, 3).reshape(
            G, N, OUT))
    return mu



# revision 9
# speedup vs baseline: 1.0175x; 1.0175x over previous
"""GCN decoder kernel for Trainium2, 8-core data-parallel over graphs.

Reference computation (per graph):
    a_hat = adj + I;  deg_j = sum_i a_hat[i,j];  d = rsqrt(deg)
    x = node_feat
    for l in 3 layers:
        h  = a_norm^T @ (x @ conv_w[l]) + conv_b[l]
        h  = h @ mlp_w[l] + mlp_b[l]
        x  = relu(layernorm(h))          # ln_g=1, ln_b=0
    mu = x @ lin_w + lin_b

Restructuring (exact algebra, host-side):
  - a_norm = d_i*(adj+I)*d_j precomputed on host, quantized to fp8e4.
  - conv_w[l] @ mlp_w[l] fused into W12[l] ((A^T x W1) W2 = (A^T x)(W1 W2)):
    each layer is ONE aggregation + ONE 128x128 matmul; x stays node-major
    the whole network -> no inter-layer transposes.
  - b2[l] = conv_b@mlp_w + mlp_b added via one K=1 512-wide matmul per
    PSUM tile in the same accumulation group as the weight matmuls.
  - a_norm scaled by 2^6, x0 by 2^4 (compensated exactly inside W12) to
    keep fp8e4m3 values out of the subnormal range.  x0 is a SINGLE fp8
    pass (measured: hi+lo refinement does not move the end-to-end error,
    which is dominated by the adjacency/W12 quantization).

Device schedule (per core, 2 graphs):
  - Adjacency streams in COLUMN stripes (per 512 target nodes, 4 row-quarter
    DMAs each) so each 512-col aggregation chunk completes as soon as its
    stripe lands; copy/W12/LN-stats pipeline per chunk behind the DMA.
  - fp8 DoubleRow aggregation into two [128,1024] PSUM tiles (one 512-col
    accumulation group per chunk).
  - Per chunk: one 512-wide PSUM->SBUF bf16 copy (rotating over the Pool/
    DVE/ACT engines -- the GPSIMD Pool engine is otherwise idle), one K=1
    512-wide bias matmul + 4 W12 matmuls into a single-bank PSUM tile, LN
    stats via raw BNStats over an INTERLEAVED pair access pattern (exact
    per-block count/mean/M2 for two blocks per instruction).
  - Per half (2 chunks): stat chain (sqrt/recip/negmul) -> istd, nbias;
    then ReLU(LN) applied straight from PSUM, blocks rotated over
    ACT (1-op activation) / Pool / DVE (2-op) to balance engine load.
  - Global order g0l0, g0l1, g1l0, g0l2, g1l1, g0-final, g1l2, g1-final
    keeps the PE saturated while graph-1's adjacency still streams.
  - Final linear: g0 via per-half 3D xbar DMA-transpose, g1 (the tail)
    via PE transposes (PE idle there, lower latency); lin_b folded in as
    K=1 matmuls; per-half mu output DMAs issued as soon as ready.
"""
import numpy as np

G, N, H, OUT, L = 16, 2048, 128, 64, 3
EPS = 1e-5
N_CORES = 8
GPC = G // N_CORES          # graphs per core
NB = N // 128               # 16 node blocks
NC = 4                      # 512-col chunks per graph
NH = 2                      # 1024-column halves
ADJ_SCALE = 64.0
X0_SCALE = 16.0

_cache = {}
MARKS = []


def _build():
    import concourse.bass as bass
    import concourse.mybir as mybir
    import concourse.tile as tile
    from concourse import bacc

    f32 = mybir.dt.float32
    bf16 = mybir.dt.bfloat16
    fp8 = mybir.dt.float8e4
    Alu = mybir.AluOpType
    Act = mybir.ActivationFunctionType
    DR = mybir.MatmulPerfMode.DoubleRow

    nc = bacc.Bacc("TRN2", target_bir_lowering=False, debug=False,
                   num_devices=N_CORES)

    adjn_d = nc.dram_tensor("adjn", [GPC, N, N], fp8, kind="ExternalInput").ap()
    x0_d = nc.dram_tensor("x0", [GPC, 128, N], fp8, kind="ExternalInput").ap()
    w12_d = nc.dram_tensor("w12", [128, L * H], bf16, kind="ExternalInput").ap()
    b2rep_d = nc.dram_tensor("b2rep", [1, L * 512], bf16,
                             kind="ExternalInput").ap()
    ones1_d = nc.dram_tensor("ones1", [1, 128], bf16, kind="ExternalInput").ap()
    identb_d = nc.dram_tensor("identb", [128, 128], bf16,
                              kind="ExternalInput").ap()
    linw_d = nc.dram_tensor("lin_w", [128, OUT], bf16, kind="ExternalInput").ap()
    linb1_d = nc.dram_tensor("linb1", [1, OUT], bf16, kind="ExternalInput").ap()
    epsc_d = nc.dram_tensor("epsc", [128, 1], f32, kind="ExternalInput").ap()

    mu_d = nc.dram_tensor("mu", [GPC, 128, NB * OUT], f32,
                          kind="ExternalOutput").ap()

    with tile.TileContext(nc) as tc:
        with (
            tc.tile_pool(name="const", bufs=1) as cpool,
            tc.tile_pool(name="adjp", bufs=2 * NC) as adjp,
            tc.tile_pool(name="act", bufs=1) as act,
            tc.tile_pool(name="small", bufs=4) as small,
            tc.tile_pool(name="psA", bufs=2, space="PSUM") as psA,
            tc.tile_pool(name="psM", bufs=2, space="PSUM") as psM,
        ):
            # ---- DMA plan ----
            # gpsimd queue: x0 + consts (issues in parallel with SP).
            # SP queue: adjacency column stripes, g0 then g1; later the
            # xbar transposes and per-half mu output DMAs.
            x0s = []

            def load_x0(g, split=False):
                x0 = act.tile([128, N], fp8, tag="x0", bufs=2, name=f"x0_{g}")
                if split:   # first 2 blocks land first for agg k-step 0
                    nc.gpsimd.dma_start(x0[:, 0:256], x0_d[g][:, 0:256])
                    nc.gpsimd.dma_start(x0[:, 256:N], x0_d[g][:, 256:N])
                else:
                    nc.gpsimd.dma_start(x0[:], x0_d[g])
                x0s.append(x0)

            load_x0(0, split=True)
            ones1_t = cpool.tile([1, 128], bf16, name="ones1t")
            nc.gpsimd.dma_start(ones1_t[:], ones1_d)
            w12_t = cpool.tile([128, L * H], bf16, name="w12t")
            nc.gpsimd.dma_start(w12_t[:], w12_d)
            b2rep_t = cpool.tile([1, L * 512], bf16, name="b2rept")
            nc.gpsimd.dma_start(b2rep_t[:], b2rep_d)
            eps_t = cpool.tile([128, 1], f32, name="epst")
            nc.gpsimd.dma_start(eps_t[:], epsc_d)
            nc.const_aps.aps[(f32, EPS)] = eps_t[:]
            identb_t = cpool.tile([128, 128], bf16, name="identbt")
            nc.gpsimd.dma_start(identb_t[:], identb_d)
            linw_t = cpool.tile([128, OUT], bf16, name="linwt")
            nc.gpsimd.dma_start(linw_t[:], linw_d)
            linb1_t = cpool.tile([1, OUT], bf16, name="linb1t")
            nc.gpsimd.dma_start(linb1_t[:], linb1_d)
            load_x0(1)

            # adjacency tiles: row-quarter q holds rows q*512..q*512+511 as
            # [p, i(4), j(2048)] with row (q*512 + i*128 + p).
            adjq = [[adjp.tile([128, 4 * N], fp8, tag="adj",
                               name=f"adj_{g}_{q}") for q in range(4)]
                    for g in range(GPC)]

            def adj_stripe_dma(g, q, c, i0=0, i1=4):
                dst = adjq[g][q][:].rearrange("p (i j) -> p i j", i=4)[
                    :, i0:i1, c * 512:(c + 1) * 512]
                src = adjn_d[g, q * 512 + i0 * 128:q * 512 + i1 * 128,
                             c * 512:(c + 1) * 512].rearrange(
                    "(i p) j -> p i j", p=128)
                nc.sync.dma_start(dst, src)

            # g0 stripes (first q-tile of stripe 0 split for early start),
            # then g1 stripes.
            adj_stripe_dma(0, 0, 0, 0, 2)
            adj_stripe_dma(0, 0, 0, 2, 4)
            for q in range(1, 4):
                adj_stripe_dma(0, q, 0)
            for c in range(1, NC):
                for q in range(4):
                    adj_stripe_dma(0, q, c)
            for c in range(NC):
                for q in range(4):
                    adj_stripe_dma(1, q, c)

            def adj_pair_ap(g, t, c):
                """[128, 2, 512] slice for k-tile pair (2t, 2t+1), chunk c."""
                q, p = t // 2, t % 2
                return adjq[g][q][:].rearrange("p (i j) -> p i j", i=4)[
                    :, 2 * p:2 * p + 2, c * 512:(c + 1) * 512]

            # per-graph state
            st = [dict(y=None, aggps=None, aggT=None, ynext=None,
                       istd=None, nbias=None, h2c=[None] * NC,
                       bn6=None) for _ in range(GPC)]

            def alloc_layer(g, l):
                s = st[g]
                s["aggT"] = act.tile([128, N], bf16, tag="aggT", bufs=3,
                                     name=f"aggT_{g}_{l}")
                if l < L - 1:
                    s["ynext"] = act.tile([128, N], fp8, tag="y", bufs=4,
                                          name=f"y_{g}_{l}")
                else:
                    s["ynext"] = act.tile([128, N], bf16, tag="x3", bufs=2,
                                          name=f"x3_{g}")
                s["istd"] = small.tile([128, NB], f32, tag="istd",
                                       name=f"istd_{g}_{l}")
                s["nbias"] = small.tile([128, NB], f32, tag="nbias",
                                        name=f"nbias_{g}_{l}")
                s["bn6"] = small.tile([128, 2 * NC, 6], f32, tag="bn6",
                                      name=f"bn6_{g}_{l}")
                s["aggps"] = [psA.tile([128, 1024], f32, tag="agg",
                                       name=f"aggps_{g}_{l}_{h}")
                              for h in range(NH)]

            def emit_agg_chunk(g, l, c):
                """8 DoubleRow fp8 matmuls accumulating 512-col chunk c."""
                s = st[g]
                src = s["y"] if l > 0 else x0s[g]
                tgt = s["aggps"][c // 2][:, (c % 2) * 512:(c % 2) * 512 + 512]
                for t in range(NB // 2):
                    nc.tensor.matmul(
                        tgt,
                        src[:, 2 * t * 128:(2 * t + 2) * 128].rearrange(
                            "p (two k) -> p two k", two=2),
                        adj_pair_ap(g, t, c),
                        start=(t == 0), stop=(t == NB // 2 - 1), perf_mode=DR)

            # GPSIMD (Pool) cannot touch PSUM, so all PSUM-side work lands
            # on DVE/ACT.  Cheapest total: copies on DVE, applies as 1-op
            # activations on ACT; Pool only gets SBUF-side chain ops.
            def emit_copy(g, l, c):
                """PSUM -> SBUF bf16, one 512-wide op on DVE."""
                s = st[g]
                src = s["aggps"][c // 2][:, (c % 2) * 512:(c % 2) * 512 + 512]
                dst = s["aggT"][:, c * 512:(c + 1) * 512]
                nc.vector.tensor_copy(dst, src)

            def emit_w12(g, l, c):
                """one K=1 512-wide bias matmul + 4 W12 matmuls + stats."""
                s = st[g]
                w = w12_t[:, l * H:(l + 1) * H]
                b2 = b2rep_t[:, l * 512:(l + 1) * 512]
                h2 = psM.tile([128, 512], f32, tag=f"h2{c % 2}",
                              name=f"h2_{g}_{l}_{c}")
                s["h2c"][c] = h2
                nc.tensor.matmul(h2[:], ones1_t[:], b2, start=True,
                                 stop=False, skip_group_check=True)
                for jj in range(4):
                    sl = slice(jj * 128, (jj + 1) * 128)
                    j = 4 * c + jj
                    nc.tensor.matmul(
                        h2[:, sl], s["aggT"][:, j * 128:(j + 1) * 128],
                        w, start=False, stop=(jj == 3),
                        skip_group_check=True)
                for t in range(2):  # exact per-block stats for pair of blocks
                    in_ap = h2[:, 2 * t * 128:(2 * t + 2) * 128].rearrange(
                        "p (two k) -> p k two", two=2)
                    nc.vector.add_instruction(
                        mybir.InstBNStats(
                            name=nc.get_next_instruction_name(),
                            ins=[nc.vector.lower_ap(in_ap)],
                            outs=[nc.vector.lower_ap(
                                s["bn6"][:, 2 * c + t, :])]))

            def emit_chain(g, l, h):
                """LN stat chain for half h -> istd, nbias columns."""
                s = st[g]
                # bn6[p, pair, (even triple, odd triple)]; triples are
                # (count, mean, count*var); count == 128 per block.
                tri = s["bn6"][:].rearrange("p f (t s) -> p (f t) s", t=2)
                slc = slice(8 * h, 8 * h + 8)
                means = tri[:, slc, 1]
                cvars = tri[:, slc, 2]
                stdv = small.tile([128, 8], f32, tag="stdv",
                                  name=f"stdv_{g}_{l}_{h}")
                nc.scalar.activation(stdv[:], cvars, Act.Sqrt,
                                     bias=EPS, scale=1.0 / H)
                nc.vector.reciprocal(s["istd"][:, slc], stdv[:])
                nc.vector.scalar_tensor_tensor(
                    out=s["nbias"][:, slc], in0=means, scalar=-1.0,
                    in1=s["istd"][:, slc], op0=Alu.mult, op1=Alu.mult)

            APPLY_ENG = ["scalar"] * 8
            # tail layer: alternate ACT/DVE so the last apply stream is ~2x
            # shorter (both engines idle by then)
            APPLY_TAIL = ["scalar", "vector", "scalar", "vector",
                          "scalar", "vector", "scalar", "vector"]

            def emit_apply_block(g, l, j, eng):
                s = st[g]
                h2 = s["h2c"][j // 4]
                hsl = h2[:, (j % 4) * 128:(j % 4 + 1) * 128]
                ysl = s["ynext"][:, j * 128:(j + 1) * 128]
                if eng == "scalar":
                    nc.scalar.activation(
                        ysl, hsl, Act.Relu,
                        bias=s["nbias"][:, j:j + 1],
                        scale=s["istd"][:, j:j + 1])
                else:
                    e = nc.vector if eng == "vector" else nc.gpsimd
                    tmp = small.tile([128, 128], f32, tag=f"ptmp_{eng}",
                                     name=f"ptmp_{g}_{l}_{j}", bufs=3)
                    e.tensor_scalar(tmp[:], hsl, s["istd"][:, j:j + 1],
                                    s["nbias"][:, j:j + 1],
                                    op0=Alu.mult, op1=Alu.add)
                    e.tensor_scalar(ysl, tmp[:], 0.0, None, op0=Alu.max)

            def emit_applies_half(g, l, h, engs=APPLY_ENG):
                for jj in range(8):
                    emit_apply_block(g, l, 8 * h + jj, engs[jj])

            def emit_post_half(g, l, h, engs=APPLY_ENG):
                emit_chain(g, l, h)
                emit_applies_half(g, l, h, engs)

            def finish_layer(g):
                st[g]["y"] = st[g]["ynext"]

            def emit_gl(g, l, mark_pref=None, engs=APPLY_ENG):
                """Full graph-layer: per-chunk agg/copy/W12/stats, then
                per-half chain + applies."""
                alloc_layer(g, l)
                for c in range(NC):
                    emit_agg_chunk(g, l, c)
                    emit_copy(g, l, c)
                    emit_w12(g, l, c)
                    if c == 1:
                        emit_post_half(g, l, 0, engs)
                emit_post_half(g, l, 1, engs)
                if mark_pref:
                    mark(mark_pref)
                finish_layer(g)

            # ---- finals ----
            FIN_COPY = ["vector", "scalar", "vector", "scalar",
                        "vector", "scalar", "vector", "scalar"]

            def emit_final_half(g, x3, xT, musb, h, pe_tr):
                if pe_tr:
                    for jj in range(8):
                        j = 8 * h + jj
                        sl = slice(j * 128, (j + 1) * 128)
                        trp = psA.tile([128, 128], bf16, tag="agg",
                                       name=f"trp_{g}_{j}")
                        nc.tensor.transpose(trp[:], x3[:, sl], identb_t[:])
                        if FIN_COPY[jj] == "scalar":
                            nc.scalar.copy(xT[:, sl], trp[:])
                        else:
                            nc.vector.tensor_copy(xT[:, sl], trp[:])
                else:
                    nc.sync.dma_start_transpose(
                        xT[:].rearrange("p (b q) -> p b q", b=NB)[
                            :, 8 * h:8 * h + 8, :],
                        x3[:, h * 1024:(h + 1) * 1024])
                for jj in range(8):
                    j = 8 * h + jj
                    sl = slice(j * 128, (j + 1) * 128)
                    mup = psA.tile([128, OUT], f32, tag="agg",
                                   name=f"mup_{g}_{j}")
                    nc.tensor.matmul(mup[:], xT[:, sl], linw_t[:],
                                     start=True, stop=False,
                                     skip_group_check=True)
                    nc.tensor.matmul(mup[:], ones1_t[:], linb1_t[:],
                                     start=False, stop=True,
                                     skip_group_check=True)
                    osl = slice(j * OUT, (j + 1) * OUT)
                    if FIN_COPY[jj] == "scalar":
                        nc.scalar.copy(musb[:, osl], mup[:])
                    else:
                        nc.vector.tensor_copy(musb[:, osl], mup[:])
                nc.sync.dma_start(mu_d[g][:, h * 8 * OUT:(h + 1) * 8 * OUT],
                                  musb[:, h * 8 * OUT:(h + 1) * 8 * OUT])

            MARKS.clear()

            def mark(label):
                MARKS.append((label, list(nc.all_instructions())[-1].name))

            # ---- global schedule ----
            emit_gl(0, 0, "g0l0")
            emit_gl(0, 1, "g0l1")
            emit_gl(1, 0, "g1l0")
            emit_gl(0, 2, "g0l2")
            emit_gl(1, 1, "g1l1")
            # g0 final (xbar transposes; PE only does the small mu matmuls)
            fin = []
            for g in range(GPC):
                xT = act.tile([128, N], bf16, tag="xT", bufs=2, name=f"xT_{g}")
                musb = act.tile([128, NB * OUT], f32, tag="mu", bufs=2,
                                name=f"musb_{g}")
                fin.append((xT, musb))
            for h in range(NH):
                emit_final_half(0, st[0]["ynext"], fin[0][0], fin[0][1], h,
                                pe_tr=False)
            mark("g0fin")
            emit_gl(1, 2, "g1l2", engs=APPLY_TAIL)
            for h in range(NH):
                emit_final_half(1, st[1]["ynext"], fin[1][0], fin[1][1], h,
                                pe_tr=True)
            mark("g1fin")

    nc.compile()
    return nc


def kernel(node_feat, adj, conv_w, conv_b, mlp_w, mlp_b, ln_g, ln_b, lin_w,
           lin_b, **_ignored):
    import ml_dtypes
    from concourse.bass_utils import run_bass_kernel_spmd

    bf16 = ml_dtypes.bfloat16
    fp8 = ml_dtypes.float8_e4m3

    node_feat = np.asarray(node_feat, dtype=np.float32)
    adj = np.asarray(adj, dtype=np.float32)
    conv_w = np.asarray(conv_w, dtype=np.float32)
    conv_b = np.asarray(conv_b, dtype=np.float32)
    mlp_w = np.asarray(mlp_w, dtype=np.float32)
    mlp_b = np.asarray(mlp_b, dtype=np.float32)
    ln_g = np.asarray(ln_g, dtype=np.float32)
    ln_b = np.asarray(ln_b, dtype=np.float32)
    lin_w = np.asarray(lin_w, dtype=np.float32)
    lin_b = np.asarray(lin_b, dtype=np.float32)

    assert np.allclose(ln_g, 1.0) and np.allclose(ln_b, 0.0), \
        "kernel specialized for ln_g=1, ln_b=0 (as produced by setup_inputs)"

    if "nc" not in _cache:
        _cache["nc"] = _build()
    nc = _cache["nc"]

    # ---- host-side exact preprocessing ----
    deg = 1.0 + adj.sum(axis=1)                      # [G, N]
    d = deg ** -0.5
    adjn = np.empty((G, N, N), dtype=fp8)
    idx = np.arange(N)
    for g in range(G):
        an = adj[g] * (ADJ_SCALE * d[g][:, None] * d[g][None, :])
        an[idx, idx] += ADJ_SCALE * d[g] * d[g]
        adjn[g] = an.astype(fp8)

    # x0 in node-block layout [g, p, (i k)]: node (i*128+p) -> [p, i*H+k]
    x0 = node_feat.reshape(G, NB, 128, H).transpose(0, 2, 1, 3).reshape(
        G, 128, N) * X0_SCALE
    w12 = np.einsum('lhx,lxk->lhk', conv_w, mlp_w)
    w12[0] /= (ADJ_SCALE * X0_SCALE)
    w12[1] /= ADJ_SCALE
    w12[2] /= ADJ_SCALE
    w12_t = np.ascontiguousarray(
        w12.transpose(1, 0, 2).reshape(H, L * H)).astype(bf16)
    b2 = np.einsum('lh,lhk->lk', conv_b, mlp_w) + mlp_b        # [L, H]
    b2rep = np.tile(b2[:, None, :], (1, 4, 1)).reshape(1, L * 512).astype(bf16)
    ones1 = np.ones((1, 128), dtype=bf16)
    identb = np.eye(128, dtype=np.float32).astype(bf16)
    linw = lin_w.astype(bf16)
    linb1 = lin_b.reshape(1, OUT).astype(bf16)
    epsc = np.full((128, 1), EPS, dtype=np.float32)

    in_maps = []
    for c in range(N_CORES):
        m = {
            "adjn": np.ascontiguousarray(adjn[c * GPC:(c + 1) * GPC]),
            "x0": np.ascontiguousarray(
                x0[c * GPC:(c + 1) * GPC].astype(fp8)),
            "w12": w12_t, "b2rep": b2rep, "ones1": ones1, "identb": identb,
            "lin_w": linw, "linb1": linb1, "epsc": epsc,
        }
        in_maps.append(m)

    res = run_bass_kernel_spmd(nc, in_maps, core_ids=list(range(N_CORES)),
                               **_cache.get("run_kwargs", {}))
    _cache["last_result"] = res
    mu_blk = np.concatenate([res.results[c]["mu"] for c in range(N_CORES)],
                            axis=0)                      # [G, 128, NB*OUT]
    mu = np.ascontiguousarray(
        mu_blk.reshape(G, 128, NB, OUT).transpose(0, 2, 1, 3).reshape(
            G, N, OUT))
    return mu
